# revision 23
# baseline (speedup 1.0000x reference)
"""Fused single-head attention (QKV projection + softmax(QK^T/8) @ V) on 8
Trainium2 NeuronCores.

Problem: x [4, 2048, 1024] f32, kernel [3, 1024, 1024] f32 ->
         out [4, 2048, 1024] f32.

Sharding: 8 cores = 4 batches x 2 query-halves. No collectives.

v2 design. Scores are computed via the fused form
    S = xq (Wq Wk^T) x^T
so the per-core matmul stream is (in 128x128x512 units, ~213ns each):
    M  = Wq Wk^T                 128 mm   (bf16 weights; runs first, tiny lead-in)
    TT = M-chunks @ xq^T         128 mm   (f32r; contraction d, out [d',q])
    S^T[k,q] = sum_d' x TT       256 mm   (x slices stationary; E=exp bf16)
    V  = x W_v                   256 mm   (f32r in, bf16 out)
    AV^T[o,q] = sum_k V E        256 mm   (V slices stationary, E moving)
  total 1024 big matmuls vs 1152 for the classic Q/K/S split, no KT buffer,
  and x is DMA'd once and stays resident (64KB/part). V and E live in bf16
  (32KB each); E and wv reuse the stack region freed by M + the M-phase
  weights (the S phase hides the wv DMA that must wait for that free).

Softmax denominators cost the PE nothing: during the S phase the (idle)
DVE accumulates Esum[p,q] = sum_kc E[p,kc,q]; one gpsimd
partition_all_reduce then folds the partition axis — and its output is
replicated across partitions, which IS the broadcast the final
column-wise multiply needs. The AV^T orientation (o on partitions, q
free) is what makes the denominator a per-column scalar; the host
re-transposes output blocks for free. This also removes the 128 tiny
[128x2] denominator matmuls of v1, whose 139ns LDWEIGHTS could not hide
behind a 5ns matmul (~17us of exposed PE stall).

DMA order = consumption order: wqT/wkT (bf16, 4MB) -> xt own-q-half chunks
-> remaining xt -> wv, so every phase's inputs land during the previous
phase's compute and the PE never waits after the ~1.5us lead-in.

Key permutation note: the host feeds xt chunks with the core's own q-half
first; V rows and S^T rows then use the same permuted key order, and the
AV contraction over k is order-invariant, so no unpermuting is needed.
"""

import numpy as np
from contextlib import ExitStack

import concourse.bacc as bacc
import concourse.bass_isa as bass_isa
import concourse.mybir as mybir
import concourse.tile as tile
from concourse.bass_utils import run_bass_kernel_spmd

F32 = mybir.dt.float32
F32R = mybir.dt.float32r
BF16 = mybir.dt.bfloat16
EXP = mybir.ActivationFunctionType.Exp

B, S, D, DO = 4, 2048, 1024, 1024
QH = S // 2        # queries per core
DC = D // 128      # d 128-chunks
OC = DO // 128     # o 128-chunks
KC = S // 128      # key 128-chunks
SS = S // 512      # 512-wide s-chunks
SCALE = 1.0 / 8.0  # 1/sqrt(64) hardcoded in the reference


def _declare_io(nc):
    xt_d = nc.dram_tensor("xt", [SS, 128, DC, 512], F32R,
                          kind="ExternalInput").ap()
    wqk_d = nc.dram_tensor("wqk", [OC, 128, D + DO], BF16,
                           kind="ExternalInput").ap()
    wv_d = nc.dram_tensor("wv", [128, DC, DO], BF16,
                          kind="ExternalInput").ap()
    xn_d = nc.dram_tensor("xn", [KC, 128, D], BF16,
                          kind="ExternalInput").ap()
    # out block idx = oc*2 + qh -> AV^T block [128 o (chunk oc), 512 q]
    out_d = nc.dram_tensor("out", [16, 128, 512], F32, kind="ExternalOutput").ap()
    return xt_d, wqk_d, wv_d, xn_d, out_d


def _attention_core(tc):
    nc = tc.nc
    xt_d, wqk_d, wv_d, xn_d, out_d = _declare_io(nc)

    with ExitStack() as ctx:
        # Persistent: TT 32KB + E 32KB + Esum 4KB (+ psum pool).
        pTT = ctx.enter_context(tc.tile_pool(name="pTT", bufs=1))
        TT = pTT.tile([128, DC, QH], F32R, tag="TT")      # TT[d', q] per d'-chunk
        # One shared PSUM pool for every phase: 8 bufs = all 8 banks, so a
        # new phase's first chain lands 8 banks behind its own casts and
        # never WAR-stalls on the previous phase's last PSUM reads.
        psum = ctx.enter_context(tc.tile_pool(name="psum", bufs=8,
                                              space="PSUM"))

        pE = ctx.enter_context(tc.tile_pool(name="pE", bufs=1))
        E = pE.tile([128, KC, QH], BF16, tag="E")         # E[k, q] per k-chunk
        pAcc = ctx.enter_context(tc.tile_pool(name="pAcc", bufs=1))
        Esum = pAcc.tile([128, QH], F32, tag="Esum")      # sum over kc of E

        pX = ctx.enter_context(tc.tile_pool(name="pX", bufs=1))
        xt = pX.tile([128, DC, S], F32R, tag="xt")

        with ExitStack() as mctx:
            # M frame: M 32KB (+ nested wqk 32KB) -> peak 192KB.
            pScr = mctx.enter_context(tc.tile_pool(name="pScr", bufs=1))
            M = pScr.tile([128, DC, DO], F32R, tag="M")   # M[d, d'] per d-chunk

            # ---- Phase M: M[d, d'] = sum_o Wq^T[o, d] Wk^T[o, d'] -----
            with ExitStack() as pc:
                wpool = pc.enter_context(tc.tile_pool(name="wqk", bufs=1))
                wqk = [wpool.tile([128, D + DO], BF16, tag=f"wqk{oc}",
                                  name=f"wqk{oc}") for oc in range(OC)]
                # One merged DMA per oc (wq cols 0:1024 | wk cols 1024:2048):
                # the first matmul fires after a single 0.5MB transfer, and
                # the ~1us-per-dma_start SP issue cost is halved.
                for oc in range(OC):
                    nc.sync.dma_start(wqk[oc], wqk_d[oc])
                for j in range(SS):
                    nc.sync.dma_start(xt[:, :, j * 512:(j + 1) * 512], xt_d[j])

                for dcd in range(DC):
                    ps0 = psum.tile([128, 512], F32, tag="ps")
                    ps1 = psum.tile([128, 512], F32, tag="ps")
                    for oc in range(OC):
                        st = wqk[oc][:, dcd * 128:(dcd + 1) * 128]
                        nc.tensor.matmul(ps0, st, wqk[oc][:, D:D + 512],
                                         start=(oc == 0), stop=(oc == OC - 1))
                        nc.tensor.matmul(ps1, st, wqk[oc][:, D + 512:D + 1024],
                                         start=(oc == 0), stop=(oc == OC - 1))
                    nc.vector.tensor_copy(out=M[:, dcd, 0:512], in_=ps0)
                    nc.vector.tensor_copy(out=M[:, dcd, 512:1024], in_=ps1)

            # ---- Phase TT: TT[d', q] = sum_d M[d, d'] xq^T[d, q] ------
            # xq = own q-half = xt cols 0..1024 (host puts own half first).
            with ExitStack() as pc:
                for dpc in range(DC):
                    ps0 = psum.tile([128, 512], F32, tag="ps")
                    ps1 = psum.tile([128, 512], F32, tag="ps")
                    for dcd in range(DC):
                        st = M[:, dcd, dpc * 128:(dpc + 1) * 128]
                        nc.tensor.matmul(ps0, st, xt[:, dcd, 0:512],
                                         start=(dcd == 0), stop=(dcd == DC - 1))
                        nc.tensor.matmul(ps1, st, xt[:, dcd, 512:1024],
                                         start=(dcd == 0), stop=(dcd == DC - 1))
                    nc.vector.tensor_copy(out=TT[:, dpc, 0:512], in_=ps0)
                    nc.vector.tensor_copy(out=TT[:, dpc, 512:1024], in_=ps1)
        # M + wqk freed: E and wv reuse that region (WAR deps end of TT).

        with ExitStack() as vc:
            # ---- Phase S: S^T[k,q] = sum_d' x[k,d'] TT[d',q]; E = exp -
            with ExitStack() as pc:
                for kc in range(KC):
                    ps0 = psum.tile([128, 512], F32, tag="ps")
                    ps1 = psum.tile([128, 512], F32, tag="ps")
                    for dpc in range(DC):
                        st = xt[:, dpc, kc * 128:(kc + 1) * 128]
                        nc.tensor.matmul(ps0, st, TT[:, dpc, 0:512],
                                         start=(dpc == 0), stop=(dpc == DC - 1))
                        nc.tensor.matmul(ps1, st, TT[:, dpc, 512:1024],
                                         start=(dpc == 0), stop=(dpc == DC - 1))
                    nc.scalar.activation(E[:, kc, 0:512], ps0, EXP, scale=SCALE)
                    nc.scalar.activation(E[:, kc, 512:1024], ps1, EXP,
                                         scale=SCALE)
                    # idle-DVE partial denominator: Esum += E[:, kc, :]
                    if kc == 0:
                        nc.vector.tensor_copy(out=Esum, in_=E[:, 0, :])
                    else:
                        nc.vector.tensor_add(Esum, Esum, E[:, kc, :])


        # P-frame tiles live ABOVE xt in fresh stack space (no WAR), so
        # the xn/wv DMAs queue right behind xt's and land mid-S-phase.
        # ---- Phase P: P[d, q] = sum_k x[k, d] E[k, q]  (bf16 out) -----
        # x in k-partition layout (xnat, same slot order as E rows),
        # DMA'd per kc chunk so the first chains start ~1.5us after the
        # region frees; dc in groups of 4 (x2 q-halves = 8 psum chains)
        # so consumption is kc-major, matching the DMA arrival order.
        with ExitStack() as pc:
            xnp = pc.enter_context(tc.tile_pool(name="xnp", bufs=1))
            xn = xnp.tile([128, KC, D], BF16, tag="xn")
            for kc in range(KC):
                nc.sync.dma_start(xn[:, kc], xn_d[kc])
            wvp = pc.enter_context(tc.tile_pool(name="wvp", bufs=1))
            wvt = wvp.tile([128, DC, DO], BF16, tag="wv")
            nc.sync.dma_start(wvt, wv_d)
            pP = pc.enter_context(tc.tile_pool(name="pP", bufs=1))
            P = pP.tile([128, DC, QH], BF16, tag="P")     # P[d, q] per d-chunk
            pAcc2 = pc.enter_context(tc.tile_pool(name="pAcc2", bufs=1))
            rec = pAcc2.tile([128, QH], F32, tag="rec")   # denom -> 1/denom
            nc.gpsimd.partition_all_reduce(rec, Esum, 128,
                                           bass_isa.ReduceOp.add)
            nc.vector.reciprocal(rec, rec)
            for g in range(2):
                pss = [psum.tile([128, 512], F32, tag="ps", name=f"psP{g}_{i}")
                       for i in range(8)]
                for kc in range(KC):
                    for i, dc in enumerate(range(g * 4, g * 4 + 4)):
                        st = xn[:, kc, dc * 128:(dc + 1) * 128]
                        nc.tensor.matmul(pss[2 * i], st, E[:, kc, 0:512],
                                         start=(kc == 0), stop=(kc == KC - 1))
                        nc.tensor.matmul(pss[2 * i + 1], st,
                                         E[:, kc, 512:1024],
                                         start=(kc == 0), stop=(kc == KC - 1))
                for i, dc in enumerate(range(g * 4, g * 4 + 4)):
                    nc.vector.tensor_copy(out=P[:, dc, 0:512], in_=pss[2 * i])
                    nc.vector.tensor_copy(out=P[:, dc, 512:1024],
                                          in_=pss[2 * i + 1])

            # ---- Phase AVT: AVT[o,q] = (sum_d Wv[d,o] P[d,q]) * rec ---
            with ExitStack() as oc_pc:
                opool = oc_pc.enter_context(tc.tile_pool(name="opool",
                                                         bufs=2))
                for oc in range(OC):
                    o0 = oc * 128
                    aps0 = psum.tile([128, 512], F32, tag="ps")
                    aps1 = psum.tile([128, 512], F32, tag="ps")
                    for dc in range(DC):
                        st = wvt[:, dc, o0:o0 + 128]
                        nc.tensor.matmul(aps0, st, P[:, dc, 0:512],
                                         start=(dc == 0), stop=(dc == DC - 1))
                        nc.tensor.matmul(aps1, st, P[:, dc, 512:1024],
                                         start=(dc == 0), stop=(dc == DC - 1))
                    ot0 = opool.tile([128, 512], F32, tag="ot")
                    nc.vector.tensor_mul(ot0, aps0, rec[:, 0:512])
                    nc.sync.dma_start(out_d[oc * 2 + 0], ot0)
                    ot1 = opool.tile([128, 512], F32, tag="ot")
                    nc.vector.tensor_mul(ot1, aps1, rec[:, 512:1024])
                    nc.sync.dma_start(out_d[oc * 2 + 1], ot1)


_NC_CACHE = None


def build_nc():
    global _NC_CACHE
    if _NC_CACHE is None:
        nc = bacc.Bacc("TRN2", target_bir_lowering=False, debug=False,
                       num_devices=8)
        with tile.TileContext(nc) as tc:
            _attention_core(tc)
        nc.compile()
        _NC_CACHE = nc
    return _NC_CACHE


def _prep_dxT(x2d):
    """[rows, 1024] -> [128, DC, rows]: t[p, dc, r] = x2d[r, dc*128+p]."""
    return np.ascontiguousarray(
        x2d.T.reshape(DC, 128, x2d.shape[0]).transpose(1, 0, 2))


def make_in_maps(x, w):
    import ml_dtypes
    bf16 = ml_dtypes.bfloat16
    # wqk[oc, p, 0:D] = Wq[d, oc*128+p]; wqk[oc, p, D:] = Wk[d', oc*128+p]
    wqk_c = np.ascontiguousarray(np.concatenate(
        [w[0].T.reshape(OC, 128, D), w[1].T.reshape(OC, 128, D)],
        axis=2)).astype(bf16)
    # wv[p, dc, o] = Wv[dc*128+p, o]  (bf16)
    wv_b = np.ascontiguousarray(
        w[2].reshape(DC, 128, DO).transpose(1, 0, 2)).astype(bf16)
    in_maps = []
    for c in range(8):
        b, h = c // 2, c % 2
        xt = _prep_dxT(x[b])                       # [128, DC, S]
        xt_c = np.ascontiguousarray(
            xt.reshape(128, DC, SS, 512).transpose(2, 0, 1, 3))
        # own q-half chunks first (kernel col-slot j <- host chunk order[j])
        order = [2 * h, 2 * h + 1, 2 * (1 - h), 2 * (1 - h) + 1]
        xt_c = np.ascontiguousarray(xt_c[order])
        # xnat[kc, p, d] = x[perm(kc*128+p), d]: keys in the same slot
        # order as xt (own q-half first), k on partitions
        xp = x[b].reshape(SS, 512, D)[order].reshape(S, D)
        xn_c = np.ascontiguousarray(xp.reshape(KC, 128, D)).astype(bf16)
        in_maps.append({
            "xt": xt_c, "xn": xn_c,
            "wqk": wqk_c, "wv": wv_b,
        })
    return in_maps


def assemble_out(res_list):
    out = np.empty((B, S, DO), dtype=np.float32)
    for c in range(8):
        b, h = c // 2, c % 2
        blk = res_list[c]  # [16, 128, 512] = (oc, qh) AV^T blocks [o, q]
        core = blk.reshape(8, 2, 128, 512).transpose(1, 3, 0, 2)
        out[b, h * QH:(h + 1) * QH, :] = core.reshape(QH, DO)
    return out


def kernel(x, **rest):
    w = rest["kernel"]
    x = np.asarray(x, dtype=np.float32)
    w = np.asarray(w, dtype=np.float32)
    nc = build_nc()
    in_maps = make_in_maps(x, w)
    res = run_bass_kernel_spmd(nc, in_maps, list(range(8)))
    return assemble_out([res.results[c]["out"] for c in range(8)])


# revision 25
# speedup vs baseline: 1.1443x; 1.1443x over previous
"""Fused single-head attention (QKV projection + softmax(QK^T/8) @ V) on 8
Trainium2 NeuronCores.

Problem: x [4, 2048, 1024] f32, kernel [3, 1024, 1024] f32 ->
         out [4, 2048, 1024] f32.

Sharding: 8 cores = 4 batches x 2 query-halves. No collectives.

Two algebraic restructurings cut the per-core matmul count to the
FLOP-optimal 896 (= total network FLOPs / 8 cores) with no collectives:
  1. score fusion:  S = xq (Wq Wk^T) x^T   (precompute M = Wq Wk^T)
  2. late V-proj:   out = (attn x) Wv      (V never materializes)
Per-core stream (128x128x512-f32r units, 213.3ns each at full pstate):
    M  = Wq Wk^T                 128 mm   (bf16 weights; runs in DMA lead-in)
    TT = M-chunks @ xq^T         128 mm   (f32r; contraction d, out [d',q])
    S^T[k,q] = sum_d' x TT       256 mm   (x slices stationary; E=exp bf16)
    P[d,q]  = sum_k x E          256 mm   (x k-major bf16 stationary, E moving)
    AVT[o,q] = sum_d Wv P        128 mm   (wv bf16 stationary, P bf16 moving)
vs 1152 for the classic Q/K/V/S/AV split. x is DMA'd once per layout
(d-major for TT/S, k-major bf16 for P) and E/P/wv live in bf16.

Softmax denominators cost the PE nothing: during the S phase the (idle)
DVE accumulates Esum[p,q] = sum_kc E[p,kc,q]; one gpsimd
partition_all_reduce then folds the partition axis — and its output is
replicated across partitions, which IS the broadcast the final
column-wise multiply needs. The AV^T orientation (o on partitions, q
free) is what makes the denominator a per-column scalar; the host
re-transposes output blocks for free. This also removes the 128 tiny
[128x2] denominator matmuls of v1, whose 139ns LDWEIGHTS could not hide
behind a 5ns matmul (~17us of exposed PE stall).

DMA order = consumption order: wqT/wkT (bf16, 4MB) -> xt own-q-half chunks
-> remaining xt -> wv, so every phase's inputs land during the previous
phase's compute and the PE never waits after the ~1.5us lead-in.

Key permutation note: the host feeds xt chunks with the core's own q-half
first; V rows and S^T rows then use the same permuted key order, and the
AV contraction over k is order-invariant, so no unpermuting is needed.
"""

import numpy as np
from contextlib import ExitStack

import concourse.bacc as bacc
import concourse.bass_isa as bass_isa
import concourse.mybir as mybir
import concourse.tile as tile
from concourse.bass_utils import run_bass_kernel_spmd

F32 = mybir.dt.float32
F32R = mybir.dt.float32r
BF16 = mybir.dt.bfloat16
EXP = mybir.ActivationFunctionType.Exp

B, S, D, DO = 4, 2048, 1024, 1024
QH = S // 2        # queries per core
DC = D // 128      # d 128-chunks
OC = DO // 128     # o 128-chunks
KC = S // 128      # key 128-chunks
SS = S // 512      # 512-wide s-chunks
SCALE = 1.0 / 8.0  # 1/sqrt(64) hardcoded in the reference


def _declare_io(nc):
    xt_d = nc.dram_tensor("xt", [SS, 128, DC, 512], F32R,
                          kind="ExternalInput").ap()
    wqT_d = nc.dram_tensor("wqT", [OC, 128, DC, 128], BF16,
                           kind="ExternalInput").ap()
    wkT_d = nc.dram_tensor("wkT", [OC, 128, DO], BF16,
                           kind="ExternalInput").ap()
    wv_d = nc.dram_tensor("wv", [128, DC, DO], BF16,
                          kind="ExternalInput").ap()
    xn_d = nc.dram_tensor("xn", [KC, 128, D], BF16,
                          kind="ExternalInput").ap()
    # out block idx = oc*2 + qh -> AV^T block [128 o (chunk oc), 512 q]
    out_d = nc.dram_tensor("out", [16, 128, 512], F32, kind="ExternalOutput").ap()
    return xt_d, wqT_d, wkT_d, wv_d, xn_d, out_d


def _attention_core(tc):
    nc = tc.nc
    xt_d, wqT_d, wkT_d, wv_d, xn_d, out_d = _declare_io(nc)

    with ExitStack() as ctx:
        # Persistent: TT 32KB + E 32KB + Esum 4KB (+ psum pool).
        pTT = ctx.enter_context(tc.tile_pool(name="pTT", bufs=1))
        TT = pTT.tile([128, DC, QH], F32R, tag="TT")      # TT[d', q] per d'-chunk
        # One shared PSUM pool for every phase: 8 bufs = all 8 banks, so a
        # new phase's first chain lands 8 banks behind its own casts and
        # never WAR-stalls on the previous phase's last PSUM reads.
        psum = ctx.enter_context(tc.tile_pool(name="psum", bufs=8,
                                              space="PSUM"))

        pE = ctx.enter_context(tc.tile_pool(name="pE", bufs=1))
        E = pE.tile([128, KC, QH], BF16, tag="E")         # E[k, q] per k-chunk
        pAcc = ctx.enter_context(tc.tile_pool(name="pAcc", bufs=1))
        Esum = pAcc.tile([128, QH], F32, tag="Esum")      # sum over kc of E

        fX = ExitStack()
        pX = fX.enter_context(tc.tile_pool(name="pX", bufs=1))
        xt = pX.tile([128, DC, S], F32R, tag="xt")

        with ExitStack() as mctx:
            # M frame: M 32KB (+ nested wqk 32KB) -> peak 192KB.
            pScr = mctx.enter_context(tc.tile_pool(name="pScr", bufs=1))
            M = pScr.tile([128, DC, DO], F32R, tag="M")   # M[d, d'] per d-chunk

            # ---- Phase M: M[d, d'] = sum_o Wq^T[o, d] Wk^T[o, d'] -----
            with ExitStack() as pc:
                wpool = pc.enter_context(tc.tile_pool(name="wqk", bufs=1))
                wqT = [wpool.tile([128, DC, 128], BF16, tag=f"wq{oc}",
                                  name=f"wq{oc}") for oc in range(OC)]
                wkT = [wpool.tile([128, DO], BF16, tag=f"wk{oc}",
                                  name=f"wk{oc}") for oc in range(OC)]
                # DMAs in consumption order; xt after the M weights.
                nc.sync.dma_start(wqT[0], wqT_d[0])
                nc.sync.dma_start(wkT[0], wkT_d[0])
                for oc in range(1, OC):
                    nc.sync.dma_start(wqT[oc], wqT_d[oc])
                    nc.sync.dma_start(wkT[oc], wkT_d[oc])
                for j in range(SS):
                    nc.sync.dma_start(xt[:, :, j * 512:(j + 1) * 512], xt_d[j])

                for dcd in range(DC):
                    ps0 = psum.tile([128, 512], F32, tag="ps")
                    ps1 = psum.tile([128, 512], F32, tag="ps")
                    for oc in range(OC):
                        st = wqT[oc][:, dcd]
                        nc.tensor.matmul(ps0, st, wkT[oc][:, 0:512],
                                         start=(oc == 0), stop=(oc == OC - 1))
                        nc.tensor.matmul(ps1, st, wkT[oc][:, 512:1024],
                                         start=(oc == 0), stop=(oc == OC - 1))
                    nc.vector.tensor_copy(out=M[:, dcd, 0:512], in_=ps0)
                    nc.vector.tensor_copy(out=M[:, dcd, 512:1024], in_=ps1)

            # ---- Phase TT: TT[d', q] = sum_d M[d, d'] xq^T[d, q] ------
            # xq = own q-half = xt cols 0..1024 (host puts own half first).
            with ExitStack() as pc:
                for dpc in range(DC):
                    ps0 = psum.tile([128, 512], F32, tag="ps")
                    ps1 = psum.tile([128, 512], F32, tag="ps")
                    for dcd in range(DC):
                        st = M[:, dcd, dpc * 128:(dpc + 1) * 128]
                        nc.tensor.matmul(ps0, st, xt[:, dcd, 0:512],
                                         start=(dcd == 0), stop=(dcd == DC - 1))
                        nc.tensor.matmul(ps1, st, xt[:, dcd, 512:1024],
                                         start=(dcd == 0), stop=(dcd == DC - 1))
                    nc.vector.tensor_copy(out=TT[:, dpc, 0:512], in_=ps0)
                    nc.vector.tensor_copy(out=TT[:, dpc, 512:1024], in_=ps1)
        # M + wqk freed: E and wv reuse that region (WAR deps end of TT).

        with ExitStack() as vc:
            # ---- Phase S: S^T[k,q] = sum_d' x[k,d'] TT[d',q]; E = exp -
            with ExitStack() as pc:
                for kc in range(KC):
                    ps0 = psum.tile([128, 512], F32, tag="ps")
                    ps1 = psum.tile([128, 512], F32, tag="ps")
                    for dpc in range(DC):
                        st = xt[:, dpc, kc * 128:(kc + 1) * 128]
                        nc.tensor.matmul(ps0, st, TT[:, dpc, 0:512],
                                         start=(dpc == 0), stop=(dpc == DC - 1))
                        nc.tensor.matmul(ps1, st, TT[:, dpc, 512:1024],
                                         start=(dpc == 0), stop=(dpc == DC - 1))
                    nc.scalar.activation(E[:, kc, 0:512], ps0, EXP, scale=SCALE)
                    nc.scalar.activation(E[:, kc, 512:1024], ps1, EXP,
                                         scale=SCALE)
                    # idle-DVE partial denominator: Esum += E[:, kc, :]
                    if kc == 0:
                        nc.vector.tensor_copy(out=Esum, in_=E[:, 0, :])
                    else:
                        nc.vector.tensor_add(Esum, Esum, E[:, kc, :])


        fX.close()   # xt dead: frees 64KB for xnat/wv/P (WAR on S's end)

        # ---- Phase P: P[d, q] = sum_k x[k, d] E[k, q]  (bf16 out) -----
        # x in k-partition layout (xnat, same slot order as E rows),
        # DMA'd per kc chunk so the first chains start ~1.5us after the
        # region frees; dc in groups of 4 (x2 q-halves = 8 psum chains)
        # so consumption is kc-major, matching the DMA arrival order.
        with ExitStack() as pc:
            xnp = pc.enter_context(tc.tile_pool(name="xnp", bufs=1))
            xn = xnp.tile([128, KC, D], BF16, tag="xn")
            for kc in range(KC):
                nc.sync.dma_start(xn[:, kc], xn_d[kc])
            wvp = pc.enter_context(tc.tile_pool(name="wvp", bufs=1))
            wvt = wvp.tile([128, DC, DO], BF16, tag="wv")
            nc.sync.dma_start(wvt, wv_d)
            pP = pc.enter_context(tc.tile_pool(name="pP", bufs=1))
            P = pP.tile([128, DC, QH], BF16, tag="P")     # P[d, q] per d-chunk
            pAcc2 = pc.enter_context(tc.tile_pool(name="pAcc2", bufs=1))
            den = pAcc2.tile([128, QH], F32, tag="den")   # all-part denom
            rec = pAcc2.tile([128, QH], F32, tag="rec")   # 1/denom
            nc.gpsimd.partition_all_reduce(den, Esum, 128,
                                           bass_isa.ReduceOp.add)
            nc.vector.reciprocal(rec, den)
            for g in range(2):
                pss = [psum.tile([128, 512], F32, tag="ps", name=f"psP{g}_{i}")
                       for i in range(8)]
                for kc in range(KC):
                    for i, dc in enumerate(range(g * 4, g * 4 + 4)):
                        st = xn[:, kc, dc * 128:(dc + 1) * 128]
                        nc.tensor.matmul(pss[2 * i], st, E[:, kc, 0:512],
                                         start=(kc == 0), stop=(kc == KC - 1))
                        nc.tensor.matmul(pss[2 * i + 1], st,
                                         E[:, kc, 512:1024],
                                         start=(kc == 0), stop=(kc == KC - 1))
                for i, dc in enumerate(range(g * 4, g * 4 + 4)):
                    nc.vector.tensor_copy(out=P[:, dc, 0:512], in_=pss[2 * i])
                    nc.vector.tensor_copy(out=P[:, dc, 512:1024],
                                          in_=pss[2 * i + 1])

            # ---- Phase AVT: AVT[o,q] = (sum_d Wv[d,o] P[d,q]) * rec ---
            with ExitStack() as oc_pc:
                opool = oc_pc.enter_context(tc.tile_pool(name="opool",
                                                         bufs=4))
                for oc in range(OC):
                    o0 = oc * 128
                    aps0 = psum.tile([128, 512], F32, tag="ps")
                    aps1 = psum.tile([128, 512], F32, tag="ps")
                    for dc in range(DC):
                        st = wvt[:, dc, o0:o0 + 128]
                        nc.tensor.matmul(aps0, st, P[:, dc, 0:512],
                                         start=(dc == 0), stop=(dc == DC - 1))
                        nc.tensor.matmul(aps1, st, P[:, dc, 512:1024],
                                         start=(dc == 0), stop=(dc == DC - 1))
                    ot0 = opool.tile([128, 512], F32, tag="ot")
                    nc.vector.tensor_mul(ot0, aps0, rec[:, 0:512])
                    nc.sync.dma_start(out_d[oc * 2 + 0], ot0)
                    ot1 = opool.tile([128, 512], F32, tag="ot")
                    nc.vector.tensor_mul(ot1, aps1, rec[:, 512:1024])
                    nc.sync.dma_start(out_d[oc * 2 + 1], ot1)


_NC_CACHE = None


def build_nc():
    global _NC_CACHE
    if _NC_CACHE is None:
        nc = bacc.Bacc("TRN2", target_bir_lowering=False, debug=False,
                       num_devices=8)
        with tile.TileContext(nc) as tc:
            _attention_core(tc)
        nc.compile()
        _NC_CACHE = nc
    return _NC_CACHE


def _prep_dxT(x2d):
    """[rows, 1024] -> [128, DC, rows]: t[p, dc, r] = x2d[r, dc*128+p]."""
    return np.ascontiguousarray(
        x2d.T.reshape(DC, 128, x2d.shape[0]).transpose(1, 0, 2))


def make_in_maps(x, w):
    import ml_dtypes
    bf16 = ml_dtypes.bfloat16
    # wqT[oc, p, dcd, dd] = Wq[dcd*128+dd, oc*128+p]
    wqT_c = np.ascontiguousarray(
        w[0].T.reshape(OC, 128, DC, 128)).astype(bf16)
    # wkT[oc, p, d'] = Wk[d', oc*128+p]
    wkT_c = np.ascontiguousarray(w[1].T.reshape(OC, 128, D)).astype(bf16)
    # wv[p, dc, o] = Wv[dc*128+p, o]  (bf16)
    wv_b = np.ascontiguousarray(
        w[2].reshape(DC, 128, DO).transpose(1, 0, 2)).astype(bf16)
    in_maps = []
    for c in range(8):
        b, h = c // 2, c % 2
        xt = _prep_dxT(x[b])                       # [128, DC, S]
        xt_c = np.ascontiguousarray(
            xt.reshape(128, DC, SS, 512).transpose(2, 0, 1, 3))
        # own q-half chunks first (kernel col-slot j <- host chunk order[j])
        order = [2 * h, 2 * h + 1, 2 * (1 - h), 2 * (1 - h) + 1]
        xt_c = np.ascontiguousarray(xt_c[order])
        # xnat[kc, p, d] = x[perm(kc*128+p), d]: keys in the same slot
        # order as xt (own q-half first), k on partitions
        xp = x[b].reshape(SS, 512, D)[order].reshape(S, D)
        xn_c = np.ascontiguousarray(xp.reshape(KC, 128, D)).astype(bf16)
        in_maps.append({
            "xt": xt_c, "xn": xn_c,
            "wqT": wqT_c, "wkT": wkT_c, "wv": wv_b,
        })
    return in_maps


def assemble_out(res_list):
    out = np.empty((B, S, DO), dtype=np.float32)
    for c in range(8):
        b, h = c // 2, c % 2
        blk = res_list[c]  # [16, 128, 512] = (oc, qh) AV^T blocks [o, q]
        core = blk.reshape(8, 2, 128, 512).transpose(1, 3, 0, 2)
        out[b, h * QH:(h + 1) * QH, :] = core.reshape(QH, DO)
    return out


def kernel(x, **rest):
    w = rest["kernel"]
    x = np.asarray(x, dtype=np.float32)
    w = np.asarray(w, dtype=np.float32)
    nc = build_nc()
    in_maps = make_in_maps(x, w)
    res = run_bass_kernel_spmd(nc, in_maps, list(range(8)))
    return assemble_out([res.results[c]["out"] for c in range(8)])
